# revision 5
# baseline (speedup 1.0000x reference)
"""Trainium2 Bass kernel for nn_Adapter2 (dense_cnn).

Strategy (8 NeuronCores, data-parallel over the clip dim B, zero collectives):
  Each core takes 32 of the 256 N-columns (2 clips x 16 frames). Host-side we
  pre-transpose x to channel-major xT [768, 197*32] per core and cast to bf16,
  so every DMA is contiguous and the contraction dim lands on partitions.

  Per core the whole module collapses into two matmul stages:
    A: y[576, cols] = [fc1_w | mlp_in_w | off_fc1_w]^T @ xT      (bf16, PSUM f32)
    elementwise middle (reads PSUM directly):
      - temporal depthwise conv (3 taps) on the fc1 block  -> z rows 0..192
      - quickGELU via ACT Silu(1.702*y+1.702*b) (1/1.702 folded into W2) -> rows 192..384
      - temporal diff into a zero-guarded padded buffer, then 3x3 spatial
        depthwise conv as 9 accumulating shifted-AP taps -> rows 384..576
      - two aux rows (all-ones, l>=1 indicator) carry all output-side biases
    B: outT[768, cols] = Wcat2[578, 768]^T @ z   (single PSUM accumulation sums
       all three branches + biases), evict bf16, DMA out.

  Token tiles follow spatial h-rows (14 l-rows = one row of the 14x14 grid),
  so the spatial conv taps are affine shifted APs into the padded buffer.
"""
import sys

if "/opt/trn_rl_repo" not in sys.path:
    sys.path.insert(0, "/opt/trn_rl_repo")

import numpy as np
import ml_dtypes

import concourse.bass as bass
import concourse.mybir as mybir
from concourse.tile import TileContext
from concourse import bass_utils, bacc

F32 = mybir.dt.float32
BF16 = mybir.dt.bfloat16
AF = mybir.ActivationFunctionType
OP = mybir.AluOpType

C = 768
CA = 192
L = 197
T = 16
NCORES = 8
NL = 32                      # N-columns per core
NCOLS = L * NL               # 6304
HGRID = 14
PADW = 16                    # padded grid row stride (w in -1..14)
PPAD_COLS = 16 * PADW * NL   # (h in -1..14) x (w in -1..14) x NL = 8192

# token tiles: tile 0 = l 0..14 (CLS + h-row 0), tiles 1..13 = h-rows 1..13
TILES = [(0, 480)] + [(480 + 448 * (k - 1), 448) for k in range(1, 14)]

NSCAL = 37
TAPS = [(dh, dw) for dh in (-1, 0, 1) for dw in (-1, 0, 1)]


def _pack_scalars(conv_w, conv_b, fc1_b, mlp_in_b, off_fc1_b, off_conv_w, off_conv_b):
    """Per-partition scalar pack [128, NSCAL] f32, partition-aligned per op."""
    s = np.zeros((128, NSCAL), np.float32)
    w0, w1, w2 = conv_w[:, 0, 0], conv_w[:, 0, 1], conv_w[:, 0, 2]
    wsum_b = (w0 + w1 + w2) * fc1_b + conv_b
    # conv_t chunk A (ca 0..128, rows 0..128)
    s[:, 0] = w0[:128]; s[:, 1] = w1[:128]; s[:, 2] = w2[:128]
    s[:, 3] = wsum_b[:128]
    s[:, 4] = (-w0 * fc1_b)[:128]; s[:, 5] = (-w2 * fc1_b)[:128]
    # conv_t chunk B (ca 128..192, rows 0..64)
    s[:64, 6] = w0[128:]; s[:64, 7] = w1[128:]; s[:64, 8] = w2[128:]
    s[:64, 9] = wsum_b[128:]
    s[:64, 10] = (-w0 * fc1_b)[128:]; s[:64, 11] = (-w2 * fc1_b)[128:]
    # gelu biases (scaled by 1.702; Silu(1.702 g + 1.702 b) = 1.702 qgelu(g+b))
    s[64:, 12] = 1.702 * mlp_in_b[0:64]
    s[:, 13] = 1.702 * mlp_in_b[64:192]
    # off_fc1 bias
    s[:, 14] = off_fc1_b[:128]
    s[:64, 15] = off_fc1_b[128:]
    # spatial taps
    wsp = off_conv_w[:, 0, 0, :, :]  # (CA, 3, 3)
    for t_i, (dh, dw) in enumerate(TAPS):
        s[:, 16 + t_i] = wsp[:128, dh + 1, dw + 1]
        s[:64, 25 + t_i] = wsp[128:, dh + 1, dw + 1]
    s[:, 34] = off_conv_b[:128]
    s[:64, 35] = off_conv_b[128:]
    s[64, 36] = 1.0   # aux1 keep
    s[65, 36] = 0.0   # aux2 zeroed on CLS cols
    return s


def build_kernel():
    nc = bacc.Bacc("TRN2", target_bir_lowering=False, debug=False,
                   num_devices=NCORES)
    xt_d = nc.declare_dram_parameter("xt", [C, NCOLS], BF16, isOutput=False)
    w1_d = nc.declare_dram_parameter("w1", [C, 576], BF16, isOutput=False)
    w2_d = nc.declare_dram_parameter("w2", [578, C], BF16, isOutput=False)
    sc_d = nc.declare_dram_parameter("scal", [128, NSCAL], F32, isOutput=False)
    out_d = nc.declare_dram_parameter("out", [C, NCOLS], BF16, isOutput=True)

    # z chunk partition sizes (576 rows + 2 aux packed as 4x128 + 66)
    ZP = [128, 128, 128, 128, 66]

    with TileContext(nc) as tc:
        with (
            tc.tile_pool(name="const", bufs=1) as cpool,
            tc.tile_pool(name="xin", bufs=1) as xpool,
            tc.tile_pool(name="z", bufs=2) as zpool,
            tc.tile_pool(name="osb", bufs=4) as opool_sb,
            tc.tile_pool(name="ypsum", bufs=1, space="PSUM") as ypool,
            tc.tile_pool(name="opsum", bufs=3, space="PSUM") as opool,
        ):
            # --- persistent constants ---
            w1_sb = []
            for i in range(6):
                t = cpool.tile([128, 576], BF16, name=f"w1_{i}")
                nc.sync.dma_start(out=t[:], in_=w1_d[i * 128:(i + 1) * 128, :])
                w1_sb.append(t)
            w2_sb = []
            for kk in range(5):
                p0 = kk * 128
                pn = ZP[kk]
                t = cpool.tile([pn, C], BF16, name=f"w2_{kk}")
                nc.sync.dma_start(out=t[:], in_=w2_d[p0:p0 + pn, :])
                w2_sb.append(t)
            scal = cpool.tile([128, NSCAL], F32, name="scal")
            nc.sync.dma_start(out=scal[:], in_=sc_d[:])

            ppad_a = cpool.tile([128, PPAD_COLS], BF16, name="ppad_a")
            ppad_b = cpool.tile([64, PPAD_COLS], BF16, name="ppad_b")
            nc.gpsimd.memset(ppad_a[:], 0.0)
            nc.gpsimd.memset(ppad_b[:], 0.0)

            # --- stream in x tiles (contiguous row slices of xT) ---
            xt_sb = []
            for k, (c0, w) in enumerate(TILES):
                chunks = []
                for i in range(6):
                    t = xpool.tile([128, w], BF16, name=f"xt_{k}_{i}")
                    nc.sync.dma_start(out=t[:], in_=xt_d[i * 128:(i + 1) * 128,
                                                         c0:c0 + w])
                    chunks.append(t)
                xt_sb.append(chunks)

            def col(j, rows=128):
                return scal[0:rows, j:j + 1]

            def colr(r0, r1, j):
                return scal[r0:r1, j:j + 1]

            z_tiles = [None] * 14   # per tile: [zc0..zc4]

            def emit_A_and_middle(k):
                c0, w = TILES[k]
                nl = w // NL  # l rows in tile (15 or 14)
                # ---- matmul A: y chunks = Wcat1 col blocks ----
                ys = []
                for m in range(5):
                    m0 = m * 128
                    mw = min(128, 576 - m0)
                    yt = ypool.tile([mw, w], F32, name=f"y{m}")
                    for i in range(6):
                        nc.tensor.matmul(yt[:, :], w1_sb[i][:, m0:m0 + mw],
                                         xt_sb[k][i][:, :],
                                         start=(i == 0), stop=(i == 5))
                    ys.append(yt)

                zc = [zpool.tile([ZP[j], 480], BF16, name=f"zc{j}")
                      for j in range(5)]
                z_tiles[k] = zc

                # views [p, l, b, t]
                def v(ap, p0, p1, width=w, off=0):
                    return ap[p0:p1, off:off + width].rearrange(
                        "p (l b t) -> p l b t", b=2, t=T)

                # ---- conv_t (main branch) ----
                for (ysrc, yp0, yp1, zdst, zp0, zp1, cw0, cw1, cw2, cs2, cf0,
                     cf15, rows) in (
                        (ys[0], 0, 128, zc[0], 0, 128, 0, 1, 2, 3, 4, 5, 128),
                        (ys[1], 0, 64, zc[1], 0, 64, 6, 7, 8, 9, 10, 11, 64)):
                    yv = v(ysrc, yp0, yp1)
                    zv = v(zdst, zp0, zp1)
                    nc.vector.tensor_scalar(
                        out=zv, in0=yv, scalar1=col(cw1, rows),
                        scalar2=col(cs2, rows), op0=OP.mult, op1=OP.add)
                    nc.vector.scalar_tensor_tensor(
                        out=zv[:, :, :, 1:], in0=yv[:, :, :, :T - 1],
                        scalar=col(cw0, rows), in1=zv[:, :, :, 1:],
                        op0=OP.mult, op1=OP.add)
                    nc.vector.scalar_tensor_tensor(
                        out=zv[:, :, :, :T - 1], in0=yv[:, :, :, 1:],
                        scalar=col(cw2, rows), in1=zv[:, :, :, :T - 1],
                        op0=OP.mult, op1=OP.add)
                    nc.vector.tensor_scalar(
                        out=zv[:, :, :, 0:1], in0=zv[:, :, :, 0:1],
                        scalar1=col(cf0, rows), scalar2=None, op0=OP.add)
                    nc.vector.tensor_scalar(
                        out=zv[:, :, :, T - 1:T], in0=zv[:, :, :, T - 1:T],
                        scalar1=col(cf15, rows), scalar2=None, op0=OP.add)

                # ---- quickGELU branch (ACT) ----
                nc.scalar.activation(zc[1][64:128, :w], ys[1][64:128, :],
                                     AF.Silu, bias=colr(64, 128, 12),
                                     scale=1.702)
                nc.scalar.activation(zc[2][0:128, :w], ys[2][0:128, :],
                                     AF.Silu, bias=colr(0, 128, 13),
                                     scale=1.702)

                # ---- temporal diff -> padded buffer (h-row k) ----
                # (a DVE op may read only one PSUM operand; evict y_off to
                # SBUF bf16 first — also enables the 2x DVE mode)
                off = 32 if k == 0 else 0     # skip CLS cols in tile 0
                padbase = ((k + 1) * PADW + 1) * NL
                yo_a = zpool.tile([128, 480], BF16, name="yo_a")
                yo_b = zpool.tile([64, 480], BF16, name="yo_b")
                nc.scalar.activation(yo_a[:, :w], ys[3][:, :], AF.Copy)
                nc.scalar.activation(yo_b[:, :w], ys[4][:, :], AF.Copy)
                for (ysrc, ppad, rows, bcol) in ((yo_a, ppad_a, 128, 14),
                                                 (yo_b, ppad_b, 64, 15)):
                    yv = v(ysrc, 0, rows, width=w - off, off=off)
                    pv = ppad[0:rows, padbase:padbase + HGRID * NL].rearrange(
                        "p (l b t) -> p l b t", b=2, t=T)
                    nc.vector.scalar_tensor_tensor(
                        out=pv[:, :, :, 1:], in0=yv[:, :, :, 1:],
                        scalar=col(bcol, rows), in1=yv[:, :, :, :T - 1],
                        op0=OP.add, op1=OP.subtract)
                    nc.vector.tensor_scalar(
                        out=pv[:, :, :, 0:1], in0=yv[:, :, :, 0:1],
                        scalar1=0.0, scalar2=col(bcol, rows),
                        op0=OP.mult, op1=OP.add)

                # ---- aux rows (partition starts must be multiples of 32:
                # write rows 64:66 together, then scale row 65 -> 0 on the
                # CLS cols of tile 0 via a 2-partition scalar multiply) ----
                nc.vector.memset(zc[4][64:66, :w], 1.0)
                if k == 0:
                    nc.vector.tensor_scalar(
                        out=zc[4][64:66, 0:32], in0=zc[4][64:66, 0:32],
                        scalar1=colr(64, 66, 36), scalar2=None, op0=OP.mult)
                    nc.vector.memset(zc[3][0:128, 0:32], 0.0)
                    nc.vector.memset(zc[4][0:64, 0:32], 0.0)

            def emit_spconv_B_out(j):
                c0, w = TILES[j]
                off = 32 if j == 0 else 0
                zc = z_tiles[j]
                # ---- spatial conv: 9 shifted taps from the padded buffer ----
                for (ppad, zdst, rows, tap0, bcol) in (
                        (ppad_a, zc[3], 128, 16, 34),
                        (ppad_b, zc[4], 64, 25, 35)):
                    zv = zdst[0:rows, off:w].rearrange(
                        "p (l b t) -> p l b t", b=2, t=T)
                    for t_i, (dh, dw) in enumerate(TAPS):
                        src0 = ((j + 1 + dh) * PADW + 1 + dw) * NL
                        pv = ppad[0:rows, src0:src0 + HGRID * NL].rearrange(
                            "p (l b t) -> p l b t", b=2, t=T)
                        if t_i == 0:
                            nc.vector.tensor_scalar(
                                out=zv, in0=pv, scalar1=col(tap0 + t_i, rows),
                                scalar2=col(bcol, rows),
                                op0=OP.mult, op1=OP.add)
                        else:
                            nc.vector.scalar_tensor_tensor(
                                out=zv, in0=pv, scalar=col(tap0 + t_i, rows),
                                in1=zv, op0=OP.mult, op1=OP.add)

                # ---- matmul B + eviction + store ----
                for m in range(6):
                    m0 = m * 128
                    ot = opool.tile([128, w], F32, name="ops")
                    for kk in range(5):
                        nc.tensor.matmul(ot[:, :], w2_sb[kk][:, m0:m0 + 128],
                                         zc[kk][:, :w],
                                         start=(kk == 0), stop=(kk == 4))
                    osb = opool_sb.tile([128, w], BF16, name="osb")
                    nc.scalar.activation(osb[:, :], ot[:, :], AF.Copy)
                    nc.sync.dma_start(out=out_d[m0:m0 + 128, c0:c0 + w],
                                      in_=osb[:, :])

            for k in range(14):
                emit_A_and_middle(k)
                if k >= 1:
                    emit_spconv_B_out(k - 1)
            emit_spconv_B_out(13)

    nc.compile()
    return nc


_cached = {}


def _get_kernel():
    if "nc" not in _cached:
        _cached["nc"] = build_kernel()
    return _cached["nc"]


def kernel(x, T, fc1_w, fc1_b, conv_w, conv_b, fc2_w, fc2_b,
           off_fc1_w, off_fc1_b, off_conv_w, off_conv_b, off_fc2_w, off_fc2_b,
           mlp_in_w, mlp_in_b, mlp_out_w, mlp_out_b):
    bf = ml_dtypes.bfloat16
    x = np.asarray(x, np.float32)
    to_np = lambda a: np.asarray(a, np.float32)
    (fc1_w, fc1_b, conv_w, conv_b, fc2_w, fc2_b, off_fc1_w, off_fc1_b,
     off_conv_w, off_conv_b, off_fc2_w, off_fc2_b, mlp_in_w, mlp_in_b,
     mlp_out_w, mlp_out_b) = map(to_np, (
        fc1_w, fc1_b, conv_w, conv_b, fc2_w, fc2_b, off_fc1_w, off_fc1_b,
        off_conv_w, off_conv_b, off_fc2_w, off_fc2_b, mlp_in_w, mlp_in_b,
        mlp_out_w, mlp_out_b))

    # per-core channel-major shards: (8, C, L*NL)
    xt = np.ascontiguousarray(
        x.reshape(L, NCORES, NL, C).transpose(1, 3, 0, 2).reshape(
            NCORES, C, NCOLS)).astype(bf)

    w1 = np.concatenate([fc1_w, mlp_in_w, off_fc1_w], axis=1).astype(bf)
    w2 = np.concatenate([
        fc2_w,
        mlp_out_w / 1.702,
        off_fc2_w,
        (fc2_b + mlp_out_b)[None, :],
        off_fc2_b[None, :],
    ], axis=0).astype(bf)
    scal = _pack_scalars(conv_w, conv_b, fc1_b, mlp_in_b, off_fc1_b,
                         off_conv_w, off_conv_b)

    nc = _get_kernel()
    in_maps = [{"xt": xt[i], "w1": w1, "w2": w2, "scal": scal}
               for i in range(NCORES)]
    res = bass_utils.run_bass_kernel_spmd(nc, in_maps,
                                          core_ids=list(range(NCORES)))
    _cached["last_result"] = res

    outT = np.stack([np.asarray(res.results[i]["out"]) for i in range(NCORES)])
    out = outT.astype(np.float32).reshape(NCORES, C, L, NL).transpose(
        2, 0, 3, 1).reshape(L, NCORES * NL, C)
    return np.ascontiguousarray(out)


# revision 9
# speedup vs baseline: 1.1992x; 1.1992x over previous
"""Trainium2 Bass kernel for nn_Adapter2 (dense_cnn).

Strategy (8 NeuronCores, data-parallel over the clip dim B, zero collectives):
  Each core takes 32 of the 256 N-columns (2 clips x 16 frames). Host-side we
  pre-transpose x to channel-major xT [768, 197*32] per core and cast to bf16,
  so every DMA is contiguous and the contraction dim lands on partitions.

  Per core the whole module collapses into two matmul stages:
    A: y[576, cols] = [fc1_w | mlp_in_w | off_fc1_w]^T @ xT      (bf16, PSUM f32)
    elementwise middle (reads PSUM directly):
      - temporal depthwise conv (3 taps) on the fc1 block  -> z rows 0..192
      - quickGELU via ACT Silu(1.702*y+1.702*b) (1/1.702 folded into W2) -> rows 192..384
      - temporal diff into a zero-guarded padded buffer, then 3x3 spatial
        depthwise conv as 9 accumulating shifted-AP taps -> rows 384..576
      - two aux rows (all-ones, l>=1 indicator) carry all output-side biases
    B: outT[768, cols] = Wcat2[578, 768]^T @ z   (single PSUM accumulation sums
       all three branches + biases), evict bf16, DMA out.

  Token tiles follow spatial h-rows (14 l-rows = one row of the 14x14 grid),
  so the spatial conv taps are affine shifted APs into the padded buffer.
"""
import sys

if "/opt/trn_rl_repo" not in sys.path:
    sys.path.insert(0, "/opt/trn_rl_repo")

import numpy as np
import ml_dtypes

import concourse.bass as bass
import concourse.mybir as mybir
from concourse.tile import TileContext
from concourse import bass_utils, bacc

F32 = mybir.dt.float32
BF16 = mybir.dt.bfloat16
AF = mybir.ActivationFunctionType
OP = mybir.AluOpType

C = 768
CA = 192
L = 197
T = 16
NCORES = 8
NL = 32                      # N-columns per core
NCOLS = L * NL               # 6304
HGRID = 14
PADW = 16                    # padded grid row stride (w in -1..14)
PPAD_COLS = 16 * PADW * NL   # (h in -1..14) x (w in -1..14) x NL = 8192

# token tiles: tile 0 = l 0..14 (CLS + h-row 0), tiles 1..13 = h-rows 1..13
TILES = [(0, 480)] + [(480 + 448 * (k - 1), 448) for k in range(1, 14)]

NSCAL = 37
TAPS = [(dh, dw) for dh in (-1, 0, 1) for dw in (-1, 0, 1)]


def _pack_scalars(conv_w, conv_b, fc1_b, mlp_in_b, off_fc1_b, off_conv_w, off_conv_b):
    """Per-partition scalar pack [128, NSCAL] f32, partition-aligned per op."""
    s = np.zeros((128, NSCAL), np.float32)
    w0, w1, w2 = conv_w[:, 0, 0], conv_w[:, 0, 1], conv_w[:, 0, 2]
    wsum_b = (w0 + w1 + w2) * fc1_b + conv_b
    # conv_t chunk A (ca 0..128, rows 0..128)
    s[:, 0] = w0[:128]; s[:, 1] = w1[:128]; s[:, 2] = w2[:128]
    s[:, 3] = wsum_b[:128]
    s[:, 4] = (-w0 * fc1_b)[:128]; s[:, 5] = (-w2 * fc1_b)[:128]
    # conv_t chunk B (ca 128..192, rows 0..64)
    s[:64, 6] = w0[128:]; s[:64, 7] = w1[128:]; s[:64, 8] = w2[128:]
    s[:64, 9] = wsum_b[128:]
    s[:64, 10] = (-w0 * fc1_b)[128:]; s[:64, 11] = (-w2 * fc1_b)[128:]
    # gelu biases (scaled by 1.702; Silu(1.702 g + 1.702 b) = 1.702 qgelu(g+b))
    s[64:, 12] = 1.702 * mlp_in_b[0:64]
    s[:, 13] = 1.702 * mlp_in_b[64:192]
    # off_fc1 bias
    s[:, 14] = off_fc1_b[:128]
    s[:64, 15] = off_fc1_b[128:]
    # spatial taps
    wsp = off_conv_w[:, 0, 0, :, :]  # (CA, 3, 3)
    for t_i, (dh, dw) in enumerate(TAPS):
        s[:, 16 + t_i] = wsp[:128, dh + 1, dw + 1]
        s[:64, 25 + t_i] = wsp[128:, dh + 1, dw + 1]
    s[:, 34] = off_conv_b[:128]
    s[:64, 35] = off_conv_b[128:]
    s[64, 36] = 1.0   # aux1 keep
    s[65, 36] = 0.0   # aux2 zeroed on CLS cols
    return s


def build_kernel():
    nc = bacc.Bacc("TRN2", target_bir_lowering=False, debug=False,
                   num_devices=NCORES)
    xt_d = nc.declare_dram_parameter("xt", [C, NCOLS], BF16, isOutput=False)
    w1_d = nc.declare_dram_parameter("w1", [C, 576], BF16, isOutput=False)
    w2_d = nc.declare_dram_parameter("w2", [578, C], BF16, isOutput=False)
    sc_d = nc.declare_dram_parameter("scal", [128, NSCAL], F32, isOutput=False)
    out_d = nc.declare_dram_parameter("out", [C, NCOLS], BF16, isOutput=True)

    # z chunk partition sizes (576 rows + 2 aux packed as 4x128 + 66)
    ZP = [128, 128, 128, 128, 66]

    with TileContext(nc) as tc:
        with (
            tc.tile_pool(name="const", bufs=1) as cpool,
            tc.tile_pool(name="xin", bufs=1) as xpool,
            tc.tile_pool(name="z", bufs=2) as zpool,
            tc.tile_pool(name="osb", bufs=4) as opool_sb,
            tc.tile_pool(name="ypsum", bufs=1, space="PSUM") as ypool,
            tc.tile_pool(name="opsum", bufs=3, space="PSUM") as opool,
        ):
            # --- persistent constants ---
            w1_sb = []
            for i in range(6):
                t = cpool.tile([128, 576], BF16, name=f"w1_{i}")
                nc.sync.dma_start(out=t[:], in_=w1_d[i * 128:(i + 1) * 128, :])
                w1_sb.append(t)
            w2_sb = []
            for kk in range(5):
                p0 = kk * 128
                pn = ZP[kk]
                t = cpool.tile([pn, C], BF16, name=f"w2_{kk}")
                nc.sync.dma_start(out=t[:], in_=w2_d[p0:p0 + pn, :])
                w2_sb.append(t)
            scal = cpool.tile([128, NSCAL], F32, name="scal")
            nc.sync.dma_start(out=scal[:], in_=sc_d[:])

            ppad_a = cpool.tile([128, PPAD_COLS], BF16, name="ppad_a")
            ppad_b = cpool.tile([64, PPAD_COLS], BF16, name="ppad_b")
            nc.gpsimd.memset(ppad_a[:], 0.0)
            nc.gpsimd.memset(ppad_b[:], 0.0)

            # --- stream in x tiles (contiguous row slices of xT) ---
            xt_sb = []
            for k, (c0, w) in enumerate(TILES):
                chunks = []
                for i in range(6):
                    t = xpool.tile([128, w], BF16, name=f"xt_{k}_{i}")
                    nc.sync.dma_start(out=t[:], in_=xt_d[i * 128:(i + 1) * 128,
                                                         c0:c0 + w])
                    chunks.append(t)
                xt_sb.append(chunks)

            def col(j, rows=128):
                return scal[0:rows, j:j + 1]

            def colr(r0, r1, j):
                return scal[r0:r1, j:j + 1]

            z_tiles = [None] * 14   # per tile: [zc0..zc4]

            def emit_A_and_middle(k):
                c0, w = TILES[k]
                nl = w // NL  # l rows in tile (15 or 14)
                # ---- matmul A: y chunks = Wcat1 col blocks ----
                ys = []
                for m in range(5):
                    m0 = m * 128
                    mw = min(128, 576 - m0)
                    yt = ypool.tile([mw, w], F32, name=f"y{m}")
                    for i in range(6):
                        nc.tensor.matmul(yt[:, :], w1_sb[i][:, m0:m0 + mw],
                                         xt_sb[k][i][:, :],
                                         start=(i == 0), stop=(i == 5))
                    ys.append(yt)

                zc = [zpool.tile([ZP[j], 480], BF16, name=f"zc{j}")
                      for j in range(5)]
                z_tiles[k] = zc

                # views [p, l, b, t]
                def v(ap, p0, p1, width=w, off=0):
                    return ap[p0:p1, off:off + width].rearrange(
                        "p (l b t) -> p l b t", b=2, t=T)

                # ---- conv_t (main branch) ----
                for (ysrc, yp0, yp1, zdst, zp0, zp1, cw0, cw1, cw2, cs2, cf0,
                     cf15, rows) in (
                        (ys[0], 0, 128, zc[0], 0, 128, 0, 1, 2, 3, 4, 5, 128),
                        (ys[1], 0, 64, zc[1], 0, 64, 6, 7, 8, 9, 10, 11, 64)):
                    yv = v(ysrc, yp0, yp1)
                    zv = v(zdst, zp0, zp1)
                    # center tap (+interior bias) on ACT: z = w1*y + s2
                    nc.scalar.activation(
                        zv, yv, AF.Identity,
                        bias=col(cs2, rows), scale=col(cw1, rows))
                    nc.vector.scalar_tensor_tensor(
                        out=zv[:, :, :, 1:], in0=yv[:, :, :, :T - 1],
                        scalar=col(cw0, rows), in1=zv[:, :, :, 1:],
                        op0=OP.mult, op1=OP.add)
                    nc.vector.scalar_tensor_tensor(
                        out=zv[:, :, :, :T - 1], in0=yv[:, :, :, 1:],
                        scalar=col(cw2, rows), in1=zv[:, :, :, :T - 1],
                        op0=OP.mult, op1=OP.add)
                    # t-edge bias corrections on ACT: z += const
                    nc.scalar.activation(
                        zv[:, :, :, 0:1], zv[:, :, :, 0:1], AF.Identity,
                        bias=col(cf0, rows), scale=1.0)
                    nc.scalar.activation(
                        zv[:, :, :, T - 1:T], zv[:, :, :, T - 1:T], AF.Identity,
                        bias=col(cf15, rows), scale=1.0)

                # ---- quickGELU branch (ACT) ----
                nc.scalar.activation(zc[1][64:128, :w], ys[1][64:128, :],
                                     AF.Silu, bias=colr(64, 128, 12),
                                     scale=1.702)
                nc.scalar.activation(zc[2][0:128, :w], ys[2][0:128, :],
                                     AF.Silu, bias=colr(0, 128, 13),
                                     scale=1.702)

                # ---- temporal diff -> padded buffer (h-row k) ----
                # (a DVE op may read only one PSUM operand; evict y_off to
                # SBUF bf16 first — also enables the 2x DVE mode)
                off = 32 if k == 0 else 0     # skip CLS cols in tile 0
                padbase = ((k + 1) * PADW + 1) * NL
                yo_a = zpool.tile([128, 480], BF16, name="yo_a")
                yo_b = zpool.tile([64, 480], BF16, name="yo_b")
                nc.scalar.activation(yo_a[:, :w], ys[3][:, :], AF.Copy)
                nc.scalar.activation(yo_b[:, :w], ys[4][:, :], AF.Copy)
                for (ysrc, ppad, rows, bcol) in ((yo_a, ppad_a, 128, 14),
                                                 (yo_b, ppad_b, 64, 15)):
                    yv = v(ysrc, 0, rows, width=w - off, off=off)
                    pv = ppad[0:rows, padbase:padbase + HGRID * NL].rearrange(
                        "p (l b t) -> p l b t", b=2, t=T)
                    nc.vector.scalar_tensor_tensor(
                        out=pv[:, :, :, 1:], in0=yv[:, :, :, 1:],
                        scalar=col(bcol, rows), in1=yv[:, :, :, :T - 1],
                        op0=OP.add, op1=OP.subtract)
                    # t=0: p = bias (ACT: 0*y + b)
                    nc.scalar.activation(
                        pv[:, :, :, 0:1], yv[:, :, :, 0:1], AF.Identity,
                        bias=col(bcol, rows), scale=0.0)

                # ---- aux rows (partition starts must be multiples of 32:
                # write rows 64:66 together, then scale row 65 -> 0 on the
                # CLS cols of tile 0 via a 2-partition scalar multiply) ----
                nc.gpsimd.memset(zc[4][64:66, :w], 1.0)
                if k == 0:
                    nc.vector.tensor_scalar(
                        out=zc[4][64:66, 0:32], in0=zc[4][64:66, 0:32],
                        scalar1=colr(64, 66, 36), scalar2=None, op0=OP.mult)
                    nc.gpsimd.memset(zc[3][0:128, 0:32], 0.0)
                    nc.gpsimd.memset(zc[4][0:64, 0:32], 0.0)

            def emit_spconv_B_out(j):
                c0, w = TILES[j]
                off = 32 if j == 0 else 0
                zc = z_tiles[j]
                # ---- spatial conv: 9 shifted taps from the padded buffer ----
                for (ppad, zdst, rows, tap0, bcol) in (
                        (ppad_a, zc[3], 128, 16, 34),
                        (ppad_b, zc[4], 64, 25, 35)):
                    zv = zdst[0:rows, off:w].rearrange(
                        "p (l b t) -> p l b t", b=2, t=T)
                    for t_i, (dh, dw) in enumerate(TAPS):
                        src0 = ((j + 1 + dh) * PADW + 1 + dw) * NL
                        pv = ppad[0:rows, src0:src0 + HGRID * NL].rearrange(
                            "p (l b t) -> p l b t", b=2, t=T)
                        if t_i == 0:
                            # first tap (+conv bias) on ACT: z = w*p + b
                            nc.scalar.activation(
                                zv, pv, AF.Identity,
                                bias=col(bcol, rows), scale=col(tap0 + t_i, rows))
                        else:
                            nc.vector.scalar_tensor_tensor(
                                out=zv, in0=pv, scalar=col(tap0 + t_i, rows),
                                in1=zv, op0=OP.mult, op1=OP.add)

                # ---- matmul B + eviction + store ----
                for m in range(6):
                    m0 = m * 128
                    ot = opool.tile([128, w], F32, name="ops")
                    for kk in range(5):
                        nc.tensor.matmul(ot[:, :], w2_sb[kk][:, m0:m0 + 128],
                                         zc[kk][:, :w],
                                         start=(kk == 0), stop=(kk == 4))
                    osb = opool_sb.tile([128, w], BF16, name="osb")
                    nc.scalar.activation(osb[:, :], ot[:, :], AF.Copy)
                    nc.sync.dma_start(out=out_d[m0:m0 + 128, c0:c0 + w],
                                      in_=osb[:, :])

            for k in range(14):
                emit_A_and_middle(k)
                if k >= 1:
                    emit_spconv_B_out(k - 1)
            emit_spconv_B_out(13)

    nc.compile()
    return nc


_cached = {}


def _get_kernel():
    if "nc" not in _cached:
        _cached["nc"] = build_kernel()
    return _cached["nc"]


def kernel(x, T, fc1_w, fc1_b, conv_w, conv_b, fc2_w, fc2_b,
           off_fc1_w, off_fc1_b, off_conv_w, off_conv_b, off_fc2_w, off_fc2_b,
           mlp_in_w, mlp_in_b, mlp_out_w, mlp_out_b):
    bf = ml_dtypes.bfloat16
    x = np.asarray(x, np.float32)
    to_np = lambda a: np.asarray(a, np.float32)
    (fc1_w, fc1_b, conv_w, conv_b, fc2_w, fc2_b, off_fc1_w, off_fc1_b,
     off_conv_w, off_conv_b, off_fc2_w, off_fc2_b, mlp_in_w, mlp_in_b,
     mlp_out_w, mlp_out_b) = map(to_np, (
        fc1_w, fc1_b, conv_w, conv_b, fc2_w, fc2_b, off_fc1_w, off_fc1_b,
        off_conv_w, off_conv_b, off_fc2_w, off_fc2_b, mlp_in_w, mlp_in_b,
        mlp_out_w, mlp_out_b))

    # per-core channel-major shards: (8, C, L*NL)
    xt = np.ascontiguousarray(
        x.reshape(L, NCORES, NL, C).transpose(1, 3, 0, 2).reshape(
            NCORES, C, NCOLS)).astype(bf)

    w1 = np.concatenate([fc1_w, mlp_in_w, off_fc1_w], axis=1).astype(bf)
    w2 = np.concatenate([
        fc2_w,
        mlp_out_w / 1.702,
        off_fc2_w,
        (fc2_b + mlp_out_b)[None, :],
        off_fc2_b[None, :],
    ], axis=0).astype(bf)
    scal = _pack_scalars(conv_w, conv_b, fc1_b, mlp_in_b, off_fc1_b,
                         off_conv_w, off_conv_b)

    nc = _get_kernel()
    in_maps = [{"xt": xt[i], "w1": w1, "w2": w2, "scal": scal}
               for i in range(NCORES)]
    res = bass_utils.run_bass_kernel_spmd(nc, in_maps,
                                          core_ids=list(range(NCORES)))
    _cached["last_result"] = res

    outT = np.stack([np.asarray(res.results[i]["out"]) for i in range(NCORES)])
    out = outT.astype(np.float32).reshape(NCORES, C, L, NL).transpose(
        2, 0, 3, 1).reshape(L, NCORES * NL, C)
    return np.ascontiguousarray(out)
